# revision 20
# baseline (speedup 1.0000x reference)
import sys

sys.path.insert(0, "/opt/trn_rl_repo")

import numpy as np

N = 100000
D = 32
E = 1600000
NCORES = 8
ROWS_PER_CORE = N // NCORES  # 12500
P = 128
WINDOWS = (ROWS_PER_CORE + P - 1) // P  # 98

NPAIR = N // 2          # 50000 pair-rows of [2*D] f32 (256B each)
BUCKET = 32768          # int16-addressable pairs per bucket
NPAIR_B = NPAIR - BUCKET  # 17232
CHUNK_COLS = 96       # max slot columns per compute chunk
GATHER_COLS = 8         # 8 cols * 128 = 1024 idxs per dma_gather (ring cap)


def _host_pack(edge_row, edge_col, edge_val):
    """Pack edges into per-core windowed slot grids for pair-gathers.

    Rows are lex-sorted by (bucket-A count, bucket-B count) desc so each
    128-row window needs TA_w + TB_w slot columns with little padding.
    Slot (p, g) gathers one 256B pair-row wpair[col//2]; vE/vO val planes
    mask the correct half (col%2) at multiply time.
    """
    edge_row = np.asarray(edge_row).astype(np.int64)
    edge_col = np.asarray(edge_col).astype(np.int64)
    edge_val = np.asarray(edge_val).astype(np.float32)

    pair_all = edge_col // 2
    inA_all = pair_all < BUCKET
    cA_all = np.bincount(edge_row[inA_all], minlength=N)
    cB_all = np.bincount(edge_row[~inA_all], minlength=N)
    # global count-sorted rows dealt round-robin to cores: every core sees
    # a near-identical count profile, so the cross-core window maxima stay
    # close to each core's own
    rank = np.lexsort((-cB_all, -cA_all))
    core_of_row = np.empty(N, dtype=np.int64)
    local_of_row = np.empty(N, dtype=np.int64)
    core_of_row[rank] = np.arange(N) % NCORES
    local_of_row[rank] = np.arange(N) // NCORES

    per_core = []
    for c in range(NCORES):
        m = core_of_row[edge_row] == c
        r = local_of_row[edge_row[m]]  # already count-desc order
        col = edge_col[m]
        val = edge_val[m]
        pair = col // 2
        side = col % 2
        inA = pair < BUCKET
        rows_global = rank[c::NCORES]  # local row l -> global row
        per_core.append(dict(order=rows_global, r=r, pair=pair, side=side,
                             val=val, inA=inA,
                             cA=cA_all[rows_global], cB=cB_all[rows_global]))

    # global per-window TA/TB (same program across cores)
    TA = np.zeros(WINDOWS, dtype=np.int64)
    TB = np.zeros(WINDOWS, dtype=np.int64)
    for c in range(NCORES):
        pc = per_core[c]
        for key, T in (("cA", TA), ("cB", TB)):
            pad = np.zeros(WINDOWS * P, dtype=np.int64)
            pad[:ROWS_PER_CORE] = pc[key]
            np.maximum(T, pad.reshape(WINDOWS, P).max(axis=1), out=T)

    # pack windows into chunks of <= CHUNK_COLS columns
    chunks = []  # list of (w_start, w_end) windows
    w0 = 0
    cols = 0
    for w in range(WINDOWS):
        t = int(TA[w] + TB[w])
        if cols + t > CHUNK_COLS and cols > 0:
            chunks.append((w0, w))
            w0, cols = w, 0
        cols += t
    chunks.append((w0, WINDOWS))

    # global column layout: per chunk [A-cols of its windows | B-cols]
    # col_meta: for each global column: (bucket, window, slot_index_in_window)
    colA_off = np.zeros(WINDOWS, dtype=np.int64)  # global col of window's A0
    colB_off = np.zeros(WINDOWS, dtype=np.int64)
    chunk_info = []  # (w0, w1, g0, nA, nB)
    g = 0
    for (w0, w1) in chunks:
        g0 = g
        nA = int(TA[w0:w1].sum())
        nB = int(TB[w0:w1].sum())
        a = g0
        for w in range(w0, w1):
            colA_off[w] = a
            a += int(TA[w])
        b = g0 + nA
        for w in range(w0, w1):
            colB_off[w] = b
            b += int(TB[w])
        g = g0 + nA + nB
        chunk_info.append((w0, w1, g0, nA, nB))
    G = g  # total slot columns

    NI = P * G
    idx_cols = NI // 16

    # per-column chunk map, for the chunk-local val layout
    g0_of = np.zeros(G, dtype=np.int64)
    gc_of = np.zeros(G, dtype=np.int64)
    for (w0, w1, g0, nA, nB) in chunk_info:
        g0_of[g0:g0 + nA + nB] = g0
        gc_of[g0:g0 + nA + nB] = nA + nB

    metas = []
    for c in range(NCORES):
        pc = per_core[c]
        pos = pc["r"]  # local row id == count-sorted position
        w_of = pos // P
        p_of = pos % P
        # slot index within (row, bucket): stable order
        idx_lin = np.zeros(NI, dtype=np.int16)
        vals = np.zeros((P, 2 * G), dtype=np.float32)
        for bucket in (0, 1):
            sel = pc["inA"] if bucket == 0 else ~pc["inA"]
            rr = pos[sel]
            ww = w_of[sel]
            pp = p_of[sel]
            pairs = pc["pair"][sel]
            sides = pc["side"][sel]
            vv = pc["val"][sel]
            # running slot counter per (row)
            eo = np.argsort(rr, kind="stable")
            rr_s = rr[eo]
            starts = np.searchsorted(rr_s, np.arange(ROWS_PER_CORE + 1))
            slot = np.empty(rr_s.shape, dtype=np.int64)
            slot[:] = np.arange(rr_s.size) - starts[rr_s]
            base = (colA_off if bucket == 0 else colB_off)[ww[eo]]
            gcol = base + slot
            j = gcol * P + pp[eo]
            rel = pairs[eo] - (0 if bucket == 0 else BUCKET)
            idx_lin[j] = rel.astype(np.int16)
            s = sides[eo]
            vals[pp[eo][s == 0], gcol[s == 0]] = vv[eo][s == 0]
            vals[pp[eo][s == 1], G + gcol[s == 1]] = vv[eo][s == 1]
        # idx tile [128, idx_cols]: j at [16*rep + j%16, j//16]
        idxs = np.empty((P, idx_cols), dtype=np.int16)
        wrapped = idx_lin.reshape(idx_cols, 16).T
        for rep in range(8):
            idxs[rep * 16:(rep + 1) * 16, :] = wrapped
        metas.append((idxs, vals))

    layout = dict(TA=TA, TB=TB, chunk_info=chunk_info, G=G,
                  colA_off=colA_off, colB_off=colB_off)
    return per_core, metas, layout


def _build_program(layout):
    from concourse import bacc, mybir
    import concourse.tile as tile
    from concourse.library_config import mlp

    TA, TB = layout["TA"], layout["TB"]
    chunk_info = layout["chunk_info"]
    G = layout["G"]
    colA_off, colB_off = layout["colA_off"], layout["colB_off"]
    NI = P * G

    nc = bacc.Bacc()
    wpair = nc.declare_dram_parameter("wpair", [NPAIR, 2 * D],
                                      mybir.dt.float32, isOutput=False)
    idxs = nc.declare_dram_parameter("idxs", [P, NI // 16], mybir.dt.int16,
                                     isOutput=False)
    vals = nc.declare_dram_parameter("vals", [P, 2 * G], mybir.dt.float32,
                                     isOutput=False)
    biasrep = nc.declare_dram_parameter("biasrep", [P, D], mybir.dt.float32,
                                        isOutput=False)
    out = nc.declare_dram_parameter("out", [WINDOWS * P, D], mybir.dt.float32,
                                    isOutput=True)

    with tile.TileContext(nc) as tc:
        with tc.tile_pool(name="sbuf", bufs=2) as sbuf, \
             tc.tile_pool(name="msb", bufs=1) as msb:
            nc.gpsimd.load_library(mlp)
            idx_sb = msb.tile([P, NI // 16], mybir.dt.int16)
            nc.sync.dma_start(out=idx_sb[:], in_=idxs[:])
            val_sb = msb.tile([P, 2 * G], mybir.dt.float32)
            nc.sync.dma_start(out=val_sb[:], in_=vals[:])
            bias_sb = msb.tile([P, D], mybir.dt.float32)
            nc.sync.dma_start(out=bias_sb[:], in_=biasrep[:])

            wpB = wpair[BUCKET:NPAIR, :]

            for (w0, w1, g0, nA, nB) in chunk_info:
                gc = nA + nB
                if gc == 0:
                    for w in range(w0, w1):
                        nc.sync.dma_start(out=out[w * P:(w + 1) * P, :],
                                          in_=bias_sb[:])
                    continue
                Tt = sbuf.tile([P, gc * 2 * D], mybir.dt.float32, tag="T")
                Tt3 = Tt[:].rearrange("p (g e) -> p g e", e=2 * D)
                # gathers: A block [g0, g0+nA), B block [g0+nA, g0+gc)
                for (blk0, blkn, src) in ((0, nA, wpair[:]),
                                          (nA, nB, wpB)):
                    s = 0
                    while s < blkn:
                        k = min(GATHER_COLS, blkn - s)
                        c0 = blk0 + s
                        ni = k * P
                        jcol0 = (g0 + c0) * 8  # 128/16 idx-cols per slot-col
                        nc.gpsimd.dma_gather(
                            Tt3[:, c0:c0 + k, :],
                            src,
                            idx_sb[:, jcol0:jcol0 + k * 8],
                            ni,
                            ni,
                            2 * D,
                        )
                        s += k
                # multiply: m[:, g, 0, :] = Tt[:, g, 0:D] * vE
                #           m[:, g, 1, :] = Tt[:, g, D:2D] * vO
                mm = sbuf.tile([P, gc * 2 * D], mybir.dt.float32, tag="m")
                m4 = mm[:].rearrange("p (g s d) -> p g s d", s=2, d=D)
                for s_half, vbase in ((0, 0), (1, G)):
                    vv = val_sb[:, vbase + g0:vbase + g0 + gc]
                    nc.vector.tensor_tensor(
                        out=m4[:, :, s_half, :],
                        in0=Tt3[:, :, s_half * D:(s_half + 1) * D],
                        in1=vv.unsqueeze(2).to_broadcast([P, gc, D]),
                        op=mybir.AluOpType.mult,
                    )
                # per-window reduce
                for w in range(w0, w1):
                    ta, tb = int(TA[w]), int(TB[w])
                    a0 = (colA_off[w] - g0) * 2  # in sub-slot units (32 elems)
                    b0 = (colB_off[w] - g0) * 2
                    na, nbs = 2 * ta, 2 * tb
                    if na == 0 and nbs == 0:
                        nc.sync.dma_start(out=out[w * P:(w + 1) * P, :],
                                          in_=bias_sb[:])
                        continue
                    if na == 0:
                        a0, na = b0, nbs
                        nbs = 0
                    if nbs > 0:
                        # fold B segment into the A segment, at most `na`
                        # sub-slots at a time so we never write past it
                        off = b0
                        b_rem = nbs
                        while b_rem > 0:
                            k = min(na, b_rem)
                            nc.vector.tensor_tensor(
                                out=mm[:, a0 * D:(a0 + k) * D],
                                in0=mm[:, a0 * D:(a0 + k) * D],
                                in1=mm[:, off * D:(off + k) * D],
                                op=mybir.AluOpType.add,
                            )
                            off += k
                            b_rem -= k
                    n = na
                    while n > 1:
                        lo = n // 2
                        hi = n - lo
                        nc.vector.tensor_tensor(
                            out=mm[:, a0 * D:(a0 + lo) * D],
                            in0=mm[:, a0 * D:(a0 + lo) * D],
                            in1=mm[:, (a0 + hi) * D:(a0 + n) * D],
                            op=mybir.AluOpType.add,
                        )
                        n = hi
                    nc.vector.tensor_tensor(
                        out=mm[:, a0 * D:(a0 + 1) * D],
                        in0=mm[:, a0 * D:(a0 + 1) * D],
                        in1=bias_sb[:],
                        op=mybir.AluOpType.add,
                    )
                    nc.sync.dma_start(out=out[w * P:(w + 1) * P, :],
                                      in_=mm[:, a0 * D:(a0 + 1) * D])
    nc.compile()
    global _LAST_NC
    _LAST_NC = nc
    return nc


_LAST_NC = None


def kernel(edge_row, edge_col, edge_val, weight, bias):
    from concourse.bass_utils import run_bass_kernel_spmd

    weight = np.asarray(weight).astype(np.float32)
    bias = np.asarray(bias).astype(np.float32)
    wpair = np.ascontiguousarray(weight.reshape(NPAIR, 2 * D))

    per_core, metas, layout = _host_pack(edge_row, edge_col, edge_val)
    nc = _build_program(layout)

    biasrep = np.ascontiguousarray(np.tile(bias[None, :], (P, 1)))
    in_maps = [{"wpair": wpair, "idxs": metas[c][0], "vals": metas[c][1],
                "biasrep": biasrep} for c in range(NCORES)]
    res = run_bass_kernel_spmd(nc, in_maps, list(range(NCORES)))

    out_full = np.empty((N, D), dtype=np.float32)
    for c in range(NCORES):
        oc = res.results[c]["out"]  # [WINDOWS*P, D] in sorted-row order
        order = per_core[c]["order"]  # local row l -> global row
        out_full[order, :] = oc[:ROWS_PER_CORE, :]
    return out_full


# revision 27
# speedup vs baseline: 1.0074x; 1.0074x over previous
import sys

sys.path.insert(0, "/opt/trn_rl_repo")

import numpy as np

N = 100000
D = 32
E = 1600000
NCORES = 8
ROWS_PER_CORE = N // NCORES  # 12500
P = 128
WINDOWS = (ROWS_PER_CORE + P - 1) // P  # 98

NPAIR = N // 2          # 50000 pair-rows of [2*D] f32 (256B each)
BUCKET = 32768          # int16-addressable pairs per bucket
NPAIR_B = NPAIR - BUCKET  # 17232
CHUNK_COLS = 96       # max slot columns per compute chunk
GATHER_COLS = 8         # 8 cols * 128 = 1024 idxs per dma_gather (ring cap)


def _host_pack(edge_row, edge_col, edge_val):
    """Pack edges into per-core windowed slot grids for pair-gathers.

    Rows are lex-sorted by (bucket-A count, bucket-B count) desc so each
    128-row window needs TA_w + TB_w slot columns with little padding.
    Slot (p, g) gathers one 256B pair-row wpair[col//2]; vE/vO val planes
    mask the correct half (col%2) at multiply time.
    """
    edge_row = np.asarray(edge_row).astype(np.int64)
    edge_col = np.asarray(edge_col).astype(np.int64)
    edge_val = np.asarray(edge_val).astype(np.float32)

    pair_all = edge_col // 2
    inA_all = pair_all < BUCKET
    cA_all = np.bincount(edge_row[inA_all], minlength=N)
    cB_all = np.bincount(edge_row[~inA_all], minlength=N)
    # global count-sorted rows dealt round-robin to cores: every core sees
    # a near-identical count profile, so the cross-core window maxima stay
    # close to each core's own
    rank = np.lexsort((-cB_all, -cA_all))
    core_of_row = np.empty(N, dtype=np.int64)
    local_of_row = np.empty(N, dtype=np.int64)
    core_of_row[rank] = np.arange(N) % NCORES
    local_of_row[rank] = np.arange(N) // NCORES

    per_core = []
    for c in range(NCORES):
        m = core_of_row[edge_row] == c
        r = local_of_row[edge_row[m]]  # already count-desc order
        col = edge_col[m]
        val = edge_val[m]
        pair = col // 2
        side = col % 2
        inA = pair < BUCKET
        rows_global = rank[c::NCORES]  # local row l -> global row
        per_core.append(dict(order=rows_global, r=r, pair=pair, side=side,
                             val=val, inA=inA,
                             cA=cA_all[rows_global], cB=cB_all[rows_global]))

    # global per-window TA/TB (same program across cores)
    TA = np.zeros(WINDOWS, dtype=np.int64)
    TB = np.zeros(WINDOWS, dtype=np.int64)
    for c in range(NCORES):
        pc = per_core[c]
        for key, T in (("cA", TA), ("cB", TB)):
            pad = np.zeros(WINDOWS * P, dtype=np.int64)
            pad[:ROWS_PER_CORE] = pc[key]
            np.maximum(T, pad.reshape(WINDOWS, P).max(axis=1), out=T)

    # pack windows into chunks of <= CHUNK_COLS columns
    chunks = []  # list of (w_start, w_end) windows
    w0 = 0
    cols = 0
    for w in range(WINDOWS):
        t = int(TA[w] + TB[w])
        if cols + t > CHUNK_COLS and cols > 0:
            chunks.append((w0, w))
            w0, cols = w, 0
        cols += t
    chunks.append((w0, WINDOWS))

    # global column layout: per chunk [A-cols of its windows | B-cols]
    # col_meta: for each global column: (bucket, window, slot_index_in_window)
    colA_off = np.zeros(WINDOWS, dtype=np.int64)  # global col of window's A0
    colB_off = np.zeros(WINDOWS, dtype=np.int64)
    chunk_info = []  # (w0, w1, g0, nA, nB)
    g = 0
    for (w0, w1) in chunks:
        g0 = g
        nA = int(TA[w0:w1].sum())
        nB = int(TB[w0:w1].sum())
        a = g0
        for w in range(w0, w1):
            colA_off[w] = a
            a += int(TA[w])
        b = g0 + nA
        for w in range(w0, w1):
            colB_off[w] = b
            b += int(TB[w])
        g = g0 + nA + nB
        chunk_info.append((w0, w1, g0, nA, nB))
    G = g  # total slot columns

    NI = P * G
    idx_cols = NI // 16

    # per-column chunk map, for the chunk-local val layout
    g0_of = np.zeros(G, dtype=np.int64)
    gc_of = np.zeros(G, dtype=np.int64)
    for (w0, w1, g0, nA, nB) in chunk_info:
        g0_of[g0:g0 + nA + nB] = g0
        gc_of[g0:g0 + nA + nB] = nA + nB

    metas = []
    for c in range(NCORES):
        pc = per_core[c]
        pos = pc["r"]  # local row id == count-sorted position
        w_of = pos // P
        p_of = pos % P
        # slot index within (row, bucket): stable order
        idx_lin = np.zeros(NI, dtype=np.int16)
        vals = np.zeros((P, 2 * G), dtype=np.float32)
        for bucket in (0, 1):
            sel = pc["inA"] if bucket == 0 else ~pc["inA"]
            rr = pos[sel]
            ww = w_of[sel]
            pp = p_of[sel]
            pairs = pc["pair"][sel]
            sides = pc["side"][sel]
            vv = pc["val"][sel]
            # running slot counter per (row)
            eo = np.argsort(rr, kind="stable")
            rr_s = rr[eo]
            starts = np.searchsorted(rr_s, np.arange(ROWS_PER_CORE + 1))
            slot = np.empty(rr_s.shape, dtype=np.int64)
            slot[:] = np.arange(rr_s.size) - starts[rr_s]
            base = (colA_off if bucket == 0 else colB_off)[ww[eo]]
            gcol = base + slot
            j = gcol * P + pp[eo]
            rel = pairs[eo] - (0 if bucket == 0 else BUCKET)
            idx_lin[j] = rel.astype(np.int16)
            s = sides[eo]
            vals[pp[eo][s == 0], gcol[s == 0]] = vv[eo][s == 0]
            vals[pp[eo][s == 1], G + gcol[s == 1]] = vv[eo][s == 1]
        # idx tile [128, idx_cols]: j at [16*rep + j%16, j//16]
        idxs = np.empty((P, idx_cols), dtype=np.int16)
        wrapped = idx_lin.reshape(idx_cols, 16).T
        for rep in range(8):
            idxs[rep * 16:(rep + 1) * 16, :] = wrapped
        metas.append((idxs, vals))

    layout = dict(TA=TA, TB=TB, chunk_info=chunk_info, G=G,
                  colA_off=colA_off, colB_off=colB_off)
    return per_core, metas, layout


def _build_program(layout):
    from concourse import bacc, mybir
    import concourse.tile as tile
    from concourse.library_config import mlp

    TA, TB = layout["TA"], layout["TB"]
    chunk_info = layout["chunk_info"]
    G = layout["G"]
    colA_off, colB_off = layout["colA_off"], layout["colB_off"]
    NI = P * G

    nc = bacc.Bacc()
    wpair = nc.declare_dram_parameter("wpair", [NPAIR, 2 * D],
                                      mybir.dt.float32, isOutput=False)
    idxs = nc.declare_dram_parameter("idxs", [P, NI // 16], mybir.dt.int16,
                                     isOutput=False)
    vals = nc.declare_dram_parameter("vals", [P, 2 * G], mybir.dt.float32,
                                     isOutput=False)
    biasrep = nc.declare_dram_parameter("biasrep", [P, D], mybir.dt.float32,
                                        isOutput=False)
    out = nc.declare_dram_parameter("out", [WINDOWS * P, D], mybir.dt.float32,
                                    isOutput=True)

    with tile.TileContext(nc) as tc:
        with tc.tile_pool(name="sbuf", bufs=2) as sbuf, \
             tc.tile_pool(name="msb", bufs=1) as msb:
            nc.gpsimd.load_library(mlp)
            idx_sb = msb.tile([P, NI // 16], mybir.dt.int16)
            nc.sync.dma_start(out=idx_sb[:], in_=idxs[:])
            val_sb = msb.tile([P, 2 * G], mybir.dt.float32)
            nc.sync.dma_start(out=val_sb[:], in_=vals[:])
            bias_sb = msb.tile([P, D], mybir.dt.float32)
            nc.sync.dma_start(out=bias_sb[:], in_=biasrep[:])

            wpB = wpair[BUCKET:NPAIR, :]

            for (w0, w1, g0, nA, nB) in chunk_info:
                gc = nA + nB
                if gc == 0:
                    for w in range(w0, w1):
                        nc.sync.dma_start(out=out[w * P:(w + 1) * P, :],
                                          in_=bias_sb[:])
                    continue
                Tt = sbuf.tile([P, gc * 2 * D], mybir.dt.float32, tag="T")
                Tt3 = Tt[:].rearrange("p (g e) -> p g e", e=2 * D)
                # gathers: A block [g0, g0+nA), B block [g0+nA, g0+gc)
                for (blk0, blkn, src) in ((0, nA, wpair[:]),
                                          (nA, nB, wpB)):
                    s = 0
                    while s < blkn:
                        k = min(GATHER_COLS, blkn - s)
                        c0 = blk0 + s
                        ni = k * P
                        jcol0 = (g0 + c0) * 8  # 128/16 idx-cols per slot-col
                        nc.gpsimd.dma_gather(
                            Tt3[:, c0:c0 + k, :],
                            src,
                            idx_sb[:, jcol0:jcol0 + k * 8],
                            ni,
                            ni,
                            2 * D,
                        )
                        s += k
                # multiply: m[:, g, 0, :] = Tt[:, g, 0:D] * vE
                #           m[:, g, 1, :] = Tt[:, g, D:2D] * vO
                mm = sbuf.tile([P, gc * 2 * D], mybir.dt.float32, tag="m")
                m4 = mm[:].rearrange("p (g s d) -> p g s d", s=2, d=D)
                # per bucket-block mults: the A-block product can start
                # while the B-block gathers are still in flight
                for (b0c, bnc) in ((0, nA), (nA, nB)):
                    if bnc == 0:
                        continue
                    for s_half, vbase in ((0, 0), (1, G)):
                        vv = val_sb[:, vbase + g0 + b0c:
                                    vbase + g0 + b0c + bnc]
                        nc.vector.tensor_tensor(
                            out=m4[:, b0c:b0c + bnc, s_half, :],
                            in0=Tt3[:, b0c:b0c + bnc,
                                    s_half * D:(s_half + 1) * D],
                            in1=vv.unsqueeze(2).to_broadcast([P, bnc, D]),
                            op=mybir.AluOpType.mult,
                        )
                # per-window reduce
                for w in range(w0, w1):
                    ta, tb = int(TA[w]), int(TB[w])
                    a0 = (colA_off[w] - g0) * 2  # in sub-slot units (32 elems)
                    b0 = (colB_off[w] - g0) * 2
                    na, nbs = 2 * ta, 2 * tb
                    if na == 0 and nbs == 0:
                        nc.sync.dma_start(out=out[w * P:(w + 1) * P, :],
                                          in_=bias_sb[:])
                        continue
                    if na == 0:
                        a0, na = b0, nbs
                        nbs = 0
                    if nbs > 0:
                        # fold B segment into the A segment, at most `na`
                        # sub-slots at a time so we never write past it
                        off = b0
                        b_rem = nbs
                        while b_rem > 0:
                            k = min(na, b_rem)
                            nc.vector.tensor_tensor(
                                out=mm[:, a0 * D:(a0 + k) * D],
                                in0=mm[:, a0 * D:(a0 + k) * D],
                                in1=mm[:, off * D:(off + k) * D],
                                op=mybir.AluOpType.add,
                            )
                            off += k
                            b_rem -= k
                    n = na
                    while n > 1:
                        lo = n // 2
                        hi = n - lo
                        nc.vector.tensor_tensor(
                            out=mm[:, a0 * D:(a0 + lo) * D],
                            in0=mm[:, a0 * D:(a0 + lo) * D],
                            in1=mm[:, (a0 + hi) * D:(a0 + n) * D],
                            op=mybir.AluOpType.add,
                        )
                        n = hi
                    nc.vector.tensor_tensor(
                        out=mm[:, a0 * D:(a0 + 1) * D],
                        in0=mm[:, a0 * D:(a0 + 1) * D],
                        in1=bias_sb[:],
                        op=mybir.AluOpType.add,
                    )
                    nc.sync.dma_start(out=out[w * P:(w + 1) * P, :],
                                      in_=mm[:, a0 * D:(a0 + 1) * D])
    nc.compile()
    global _LAST_NC
    _LAST_NC = nc
    return nc


_LAST_NC = None


def kernel(edge_row, edge_col, edge_val, weight, bias):
    from concourse.bass_utils import run_bass_kernel_spmd

    weight = np.asarray(weight).astype(np.float32)
    bias = np.asarray(bias).astype(np.float32)
    wpair = np.ascontiguousarray(weight.reshape(NPAIR, 2 * D))

    per_core, metas, layout = _host_pack(edge_row, edge_col, edge_val)
    nc = _build_program(layout)

    biasrep = np.ascontiguousarray(np.tile(bias[None, :], (P, 1)))
    in_maps = [{"wpair": wpair, "idxs": metas[c][0], "vals": metas[c][1],
                "biasrep": biasrep} for c in range(NCORES)]
    res = run_bass_kernel_spmd(nc, in_maps, list(range(NCORES)))

    out_full = np.empty((N, D), dtype=np.float32)
    for c in range(NCORES):
        oc = res.results[c]["out"]  # [WINDOWS*P, D] in sorted-row order
        order = per_core[c]["order"]  # local row l -> global row
        out_full[order, :] = oc[:ROWS_PER_CORE, :]
    return out_full
